# revision 1
# baseline (speedup 1.0000x reference)
"""FAST multi-head attention (p=2 Taylor linear attention) for Trainium2.

Self-contained: accepts FULL inputs q,k,v [2,16,4096,32] fp32, returns the
full output [2,16,4096,32]. Shards the 32 (b,h) pairs across 8 NeuronCores
(4 per core), one Bass/Tile kernel run SPMD via PJRT.

Per (b,h) (A0=1, A1=1, A2=0.5):
  num[n,e'] = sum_m v'[m,e'] * (A0 + A1 (q.k_m) + A2 (q.k_m)^2),  v' = [v | 1]
  out       = num[:, :32] / num[:, 32]
factorized through quadratic features with a cyclic pair cover
(gap = 16..1, descending), all matmuls in bf16 (inputs cast on host):
  k-side:  kt[m, 34+32p+d] = k_d * k_{(d+16-p)%32}  (one DVE op per
           128-row tile, negative-stride output), diag k^2 at cols 546:578
  kv:      kvt_A = v'^T [ones|pad|k|cover_hi], kvt_B = v'^T [cover_lo|diag]
  q-side:  PhiOff^T = Square(E^T qT) (PE + ScalarE), PhiD^T = A2 qT^2
  ansT    = wq^T qT + wd^T PhiD^T + wc^T PhiOff^T, + sumv bias (DVE)
The b-loop is software-pipelined: phase 3 of bh-1 interleaves with phase 1
of bh so the PE always has independent matmuls available. The device
stores ansT [34, N] (numerator rows 0:32, denominator row 32); the divide
and the final [e,n]->[n,e] transpose happen on the host. Spurious diagonal
terms from the cover are cancelled through wd = KV2dd - Hmat @ KVc.
"""
import dataclasses
import numpy as np

import concourse.bass as bass
import concourse.tile as tile
from concourse import mybir, bacc
from concourse.bass_utils import run_bass_kernel_spmd

F32 = mybir.dt.float32
BF16 = mybir.dt.bfloat16
A0, A1, A2 = 1.0, 1.0, 0.5
B, H, N, D = 2, 16, 4096, 32
NJ = 16                    # cover gaps, stored descending 16..1
F = NJ * D                 # 512 off-diagonal features
NCORES = 8
BH_PER_CORE = (B * H) // NCORES   # 4
NT = N // 128              # 32 n-tiles per (b,h)
E1 = D + 2                 # 34 output rows: 32 num + den + dup-den pad


def _host_consts():
    E = np.zeros((D, F), np.float32)
    Hm = np.zeros((D, F), np.float32)
    for jj in range(NJ):          # block jj holds gap = 16 - jj
        gap = NJ - jj
        beta2 = A2 if gap < 16 else A2 / 2.0
        beta = np.sqrt(beta2).astype(np.float32)
        c = beta2 / A2
        for d1 in range(D):
            f = jj * D + d1
            d2 = (d1 + gap) % D
            E[d1, f] += beta
            E[d2, f] += beta
            Hm[d1, f] += c
            Hm[d2, f] += c
    import ml_dtypes
    E4 = np.zeros((4, 128, F), np.float32)                   # [4, 128, 512]
    for a in range(4):
        E4[a, 32 * a:32 * a + 32, :] = E
    E4 = E4.astype(ml_dtypes.bfloat16)
    HmT = np.ascontiguousarray(
        Hm.T.reshape(4, 128, D).transpose(1, 0, 2)).astype(ml_dtypes.bfloat16)
    ident = np.eye(128, dtype=np.float32)
    return E4, HmT, ident


def _ap_free(x: bass.AP, free_ap, extra_offset=0):
    return dataclasses.replace(
        x, offset=x.offset + extra_offset, ap=[x.ap[0]] + [list(p) for p in free_ap]
    )


def build_nc():
    nc = bacc.Bacc(None, target_bir_lowering=False)
    R32 = mybir.dt.float32r

    def r(ap):
        return ap if ap.dtype == R32 else ap.bitcast(R32)

    def tr(out_ap, in_ap, ident_ap, tile_position=None):
        nc.tensor.matmul(out_ap, in_ap, ident_ap, is_transpose=True,
                         tile_position=tile_position, skip_group_check=True)

    qin = nc.declare_dram_parameter("qin", [BH_PER_CORE, N, D], BF16, isOutput=False)
    kin = nc.declare_dram_parameter("kin", [BH_PER_CORE, N, D], BF16, isOutput=False)
    vin = nc.declare_dram_parameter("vin", [BH_PER_CORE, N, D], BF16, isOutput=False)
    e4_in = nc.declare_dram_parameter("e4", [4, 128, F], BF16, isOutput=False)
    hmt_in = nc.declare_dram_parameter("hmt", [128, 4, D], BF16, isOutput=False)
    id_in = nc.declare_dram_parameter("ident", [128, 128], F32, isOutput=False)
    out = nc.declare_dram_parameter("out", [BH_PER_CORE, E1, N], F32, isOutput=True)

    SQ = mybir.ActivationFunctionType.Square
    sqrt_a2 = float(np.sqrt(A2))

    with tile.TileContext(nc) as tc:
        with (
            tc.tile_pool(name="sb_const", bufs=1) as sb_const,
            tc.tile_pool(name="sb_q", bufs=2) as sb_q,
            tc.tile_pool(name="sb_k", bufs=3) as sb_k,
            tc.tile_pool(name="sb_w", bufs=2) as sb_w,
            tc.tile_pool(name="sb_phi", bufs=4) as sb_phi,
            tc.tile_pool(name="sb_ep", bufs=3) as sb_ep,
            tc.tile_pool(name="ps_kv", bufs=1, space="PSUM") as ps_kv,
            tc.tile_pool(name="ps_u", bufs=4, space="PSUM") as ps_u,
            tc.tile_pool(name="ps_ans", bufs=2, space="PSUM") as ps_ans,
        ):
            e4zs = [sb_const.tile([128, F], BF16, name=f"e4z{a}")
                    for a in range(4)]
            for a in range(4):
                nc.sync.dma_start(out=e4zs[a][:], in_=e4_in[a])
            wq4z = [sb_const.tile([128, E1], BF16, name=f"wq4z{a}")
                    for a in range(4)]
            wd4z = [sb_const.tile([128, E1], BF16, name=f"wd4z{a}")
                    for a in range(4)]
            for a in range(4):
                nc.gpsimd.memset(wq4z[a][:], 0.0)
                nc.gpsimd.memset(wd4z[a][:], 0.0)
            hmt = sb_const.tile([128, 4, D], BF16)
            nc.sync.dma_start(out=hmt[:], in_=hmt_in[:])
            ident = sb_const.tile([128, 128], F32)
            nc.sync.dma_start(out=ident[:], in_=id_in[:])
            identb = sb_const.tile([128, 128], BF16)
            nc.vector.tensor_copy(identb[:], ident[:])
            kts = [sb_const.tile([128, 8, 578], BF16, name=f"kts{i}")
                   for i in range(3)]
            vxs = [sb_const.tile([128, 8, 34], BF16, name=f"vxs{i}")
                   for i in range(3)]
            for i in range(3):
                nc.gpsimd.memset(kts[i][:, :, 0:2], 1.0)
                nc.gpsimd.memset(vxs[i][:, :, 32:34], 1.0)
            rot = [0]

            def emit_qload_start(b):
                qv = qin[b].rearrange("(a bb p) d -> p bb a d", a=4, bb=8)
                q_sb = sb_q.tile([128, 8, 4, D], BF16, tag="q_sb",
                                 name=f"q_sb{b}")
                for a in range(4):
                    nc.sync.dma_start(out=q_sb[:, :, a, :], in_=qv[:, :, a, :])
                qtb = sb_q.tile([128, 1024], BF16, tag="qtb",
                                name=f"qtb{b}")
                return q_sb, qtb

            def emit_qload_tr(cur, i):
                b, q_sb, qtb = cur["b"], cur["q_sb"], cur["qtb"]
                for bb in (2 * i, 2 * i + 1):
                    qt_ps = ps_kv.tile([128, 128], BF16, tag="kvt_b",
                                       name=f"qt_ps{b}_{bb}")
                    tr(qt_ps[:], q_sb[:, bb, :, :], identb[:])
                    nc.scalar.copy(out=qtb[:, 128 * bb:128 * bb + 128],
                                   in_=qt_ps[:])

            def emit_qload_fin(cur):
                b = cur["b"]
                phidt = sb_q.tile([128, 1024], BF16, tag="phidt",
                                  name=f"phidt{b}")
                nc.scalar.activation(out=phidt[:], in_=cur["qtb"][:], func=SQ,
                                     scale=sqrt_a2)
                cur["phidt"] = phidt

            def emit_p1_group(cur, g):
                b = cur["b"]
                kv_ = kin[b].rearrange("(t p) d -> p t d", p=128)
                vv = vin[b].rearrange("(t p) d -> p t d", p=128)
                ts8 = slice(8 * g, 8 * g + 8)
                stg = sb_k.tile([128, 8, 48], BF16, tag="stg",
                                name=f"stg{b}_{g}")
                nc.sync.dma_start(out=stg[:, :, 0:32], in_=kv_[:, ts8, :])
                nc.sync.dma_start(out=stg[:, :, 32:48],
                                  in_=kv_[:, ts8, 0:16])
                kt = kts[rot[0] % 3]
                vx = vxs[rot[0] % 3]
                rot[0] += 1
                nc.sync.dma_start(out=kt[:, :, 2:34], in_=kv_[:, ts8, :])
                nc.sync.dma_start(out=vx[:, :, 0:32], in_=vv[:, ts8, :])
                for tt in range(8):
                    t = 8 * g + tt
                    kbase = stg[:, tt, 0:32]
                    in0 = _ap_free(kbase, [[0, NJ + 1], [1, D]])
                    in1 = _ap_free(kbase, [[1, NJ + 1], [1, D]])
                    # out col for gap j at 546-32j (diag j=0 at 546:578)
                    dst = _ap_free(kt[:, tt, 546:547],
                                   [[-D, NJ + 1], [1, D]])
                    nc.vector.tensor_mul(dst, in0, in1)
                    lhs = vx[:, tt, :]
                    st, sp = (t == 0), (t == NT - 1)
                    nc.tensor.matmul(cur["ka"][:], lhs, kt[:, tt, 0:290],
                                     start=st, stop=sp)
                    nc.tensor.matmul(cur["kb"][:], lhs, kt[:, tt, 290:578],
                                     start=st, stop=sp)

            def emit_p2(cur):
                b = cur["b"]
                a_sb = sb_w.tile([E1, 290], F32, tag="a_sb", name=f"a_sb{b}")
                nc.vector.tensor_copy(a_sb[:], cur["ka"][:])
                b_sb = sb_w.tile([E1, 288], F32, tag="b_sb", name=f"b_sb{b}")
                nc.vector.tensor_copy(b_sb[:], cur["kb"][:])

                wc = sb_w.tile([128, 4, E1], BF16, tag="wc", name=f"wc{b}")
                cov = [a_sb[:, 34:162], a_sb[:, 162:290],
                       b_sb[:, 0:128], b_sb[:, 128:256]]
                for s in range(4):
                    trc = ps_kv.tile([128, E1], F32, tag="kvt_b",
                                      name=f"trc{b}_{s}")
                    tr(trc[0:128, :], cov[s], ident[0:E1, 0:E1])
                    nc.scalar.copy(out=wc[:, s, :], in_=trc[0:128, :])

                hk = ps_kv.tile([E1, D], F32, tag="kvt_b", name=f"hk{b}")
                for s in range(4):
                    nc.tensor.matmul(hk[:], wc[:, s, :], hmt[:, s, :],
                                     start=(s == 0), stop=(s == 3))

                wdt = sb_w.tile([E1, D], F32, tag="wdt", name=f"wdt{b}")
                nc.vector.scalar_tensor_tensor(
                    out=wdt[:], in0=b_sb[:, 256:288], scalar=1.0, in1=hk[:],
                    op0=mybir.AluOpType.mult, op1=mybir.AluOpType.subtract,
                )

                wqs = sb_w.tile([32, E1], BF16, tag="wqs", name=f"wqs{b}")
                wds = sb_w.tile([32, E1], BF16, tag="wds", name=f"wds{b}")
                trq = ps_kv.tile([128, E1], F32, tag="kvt_b", name=f"trq{b}")
                trd = ps_kv.tile([128, E1], F32, tag="kvt_b", name=f"trd{b}")
                tr(trq[0:32, :], a_sb[:, 2:34], ident[0:E1, 0:E1])
                tr(trd[0:32, :], wdt[:], ident[0:E1, 0:E1])
                nc.scalar.copy(out=wqs[:], in_=trq[0:32, :])
                nc.scalar.copy(out=wds[:], in_=trd[0:32, :])
                for a in range(4):
                    nc.sync.dma_start(out=wq4z[a][32 * a:32 * a + 32, :],
                                      in_=wqs[:])
                    nc.sync.dma_start(out=wd4z[a][32 * a:32 * a + 32, :],
                                      in_=wds[:])
                cur["a_sb"] = a_sb
                cur["wc"] = wc

            def emit_p3_pair(prev, half, g2):
                b = prev["b"]
                qtb, phidt = prev["qtb"], prev["phidt"]
                wc = prev["wc"]
                ns = slice(512 * half, 512 * half + 512)
                pair = (2 * g2, 2 * g2 + 1)
                phit = [sb_phi.tile([128, 4, 512], BF16, tag="phit",
                                    name=f"phit{b}_{half}_{g2}_{ci}")
                        for ci in range(2)]
                for s in range(4):
                    for ci, a in enumerate(pair):
                        pa = slice(32 * a, 32 * a + 32)
                        u_ps = ps_u.tile([128, 512], F32, tag="u",
                                         name=f"u{b}_{half}_{g2}_{s}_{ci}")
                        nc.tensor.matmul(
                            u_ps[:],
                            e4zs[a][:, 128 * s:128 * (s + 1)],
                            qtb[:, ns])
                        nc.scalar.activation(
                            out=phit[ci][:, s, :], in_=u_ps[:],
                            func=SQ, scale=1.0)
                for ci, a in enumerate(pair):
                    pa = slice(32 * a, 32 * a + 32)
                    ansT = ps_ans.tile([E1, 512], F32, tag="ansT",
                                       name=f"ansT{b}_{half}_{g2}_{ci}")
                    nc.tensor.matmul(ansT[:], wq4z[a][:],
                                     qtb[:, ns],
                                     start=True, stop=False)
                    nc.tensor.matmul(ansT[:], wd4z[a][:],
                                     phidt[:, ns],
                                     start=False, stop=False)
                    for s in range(4):
                        nc.tensor.matmul(ansT[:], wc[:, s, :],
                                         phit[ci][:, s, :],
                                         start=False, stop=(s == 3))
                    anssb = sb_ep.tile([E1, 512], F32, tag="anssb",
                                       name=f"anssb{b}_{half}_{g2}_{ci}")
                    nc.vector.tensor_scalar_add(anssb[:], ansT[:],
                                                prev["a_sb"][:, 0:1])
                    off = 1024 * a + 512 * half
                    nc.sync.dma_start(out=out[b][:, off:off + 512],
                                      in_=anssb[:])

            # software pipeline: phase-3 of bh-1 interleaves with phase-1 of bh
            prev = None
            nxt_q = None
            for b in range(BH_PER_CORE + 1):
                cur = None
                if b < BH_PER_CORE:
                    if nxt_q is None:
                        q_sb, qtb = emit_qload_start(b)
                        cur = dict(b=b, q_sb=q_sb, qtb=qtb)
                        for i in range(4):
                            emit_qload_tr(cur, i)
                    else:
                        cur = nxt_q
                    emit_qload_fin(cur)
                    cur["ka"] = ps_kv.tile([E1, 290], F32, tag="kvt_a",
                                           name=f"kvt_a{b}")
                    cur["kb"] = ps_kv.tile([E1, 288], F32, tag="kvt_b",
                                           name=f"kvt_b{b}")
                for i in range(4):
                    if prev is not None:
                        emit_p3_pair(prev, i // 2, i % 2)
                    if cur is not None:
                        emit_p1_group(cur, i)
                if cur is not None:
                    emit_p2(cur)
                if b + 1 < BH_PER_CORE:
                    q_sb, qtb = emit_qload_start(b + 1)
                    nxt_q = dict(b=b + 1, q_sb=q_sb, qtb=qtb)
                    for i in range(4):
                        emit_qload_tr(nxt_q, i)
                prev = cur

    nc.compile()
    return nc


_NC_CACHE = None


def _get_nc():
    global _NC_CACHE
    if _NC_CACHE is None:
        _NC_CACHE = build_nc()
    return _NC_CACHE


def _in_maps(q, k, v):
    import ml_dtypes
    qf = q.reshape(B * H, N, D).astype(ml_dtypes.bfloat16)
    kf = k.reshape(B * H, N, D).astype(ml_dtypes.bfloat16)
    vf = v.reshape(B * H, N, D).astype(ml_dtypes.bfloat16)
    E4, HmT, ident = _host_consts()
    in_maps = []
    for c in range(NCORES):
        sl = slice(c * BH_PER_CORE, (c + 1) * BH_PER_CORE)
        in_maps.append({
            "qin": np.ascontiguousarray(qf[sl]),
            "kin": np.ascontiguousarray(kf[sl]),
            "vin": np.ascontiguousarray(vf[sl]),
            "e4": E4, "hmt": HmT, "ident": ident,
        })
    return in_maps


def _postprocess(res):
    outs = [res.results[c]["out"] for c in range(NCORES)]
    allt = np.concatenate(outs, axis=0).reshape(B, H, E1, N)
    num = allt[:, :, 0:D, :]
    den = allt[:, :, D:D + 1, :]
    return np.ascontiguousarray(
        (num / den).transpose(0, 1, 3, 2)).astype(np.float32)


def run_traced(q, k, v):
    q = np.ascontiguousarray(np.asarray(q, dtype=np.float32))
    k = np.ascontiguousarray(np.asarray(k, dtype=np.float32))
    v = np.ascontiguousarray(np.asarray(v, dtype=np.float32))
    nc = _get_nc()
    try:
        return run_bass_kernel_spmd(nc, _in_maps(q, k, v),
                                    core_ids=list(range(NCORES)), trace=True)
    except Exception as e:
        print("traced run failed:", e)
        return None


def kernel(q, k, v):
    q = np.ascontiguousarray(np.asarray(q, dtype=np.float32))
    k = np.ascontiguousarray(np.asarray(k, dtype=np.float32))
    v = np.ascontiguousarray(np.asarray(v, dtype=np.float32))
    assert q.shape == (B, H, N, D)
    nc = _get_nc()
    res = run_bass_kernel_spmd(nc, _in_maps(q, k, v),
                               core_ids=list(range(NCORES)))
    return _postprocess(res)


if __name__ == "__main__":
    rng = np.random.default_rng(0)
    q = rng.standard_normal((B, H, N, D), dtype=np.float32)
    k = rng.standard_normal((B, H, N, D), dtype=np.float32)
    v = rng.standard_normal((B, H, N, D), dtype=np.float32)
    o = kernel(q, k, v)
    print("ran", o.shape, o.dtype)



# revision 8
# speedup vs baseline: 1.2361x; 1.2361x over previous
"""FAST multi-head attention (p=2 Taylor linear attention) for Trainium2, v2.

Self-contained: accepts FULL inputs q,k,v [2,16,4096,32] fp32, returns the
full output [2,16,4096,32]. Shards the 32 (b,h) pairs across 8 NeuronCores
(4 per core), one Bass/Tile kernel run SPMD via PJRT.

Per (b,h) (A0=1, A1=1, A2=0.5), with v' = [v | 1]:
  out[n, :] = num / den where [num|den](n, e) = sum_f phi_f(q_n) KV[f, e],
  KV[f, e] = sum_m phi_f(k_m) v'[m, e], over an exact feature basis:
  ones(1) + linear q (32) + diag q^2 (32) + 496 off-diag pair products
  (gaps 1..15 cyclic + half gap 16). Off-diag phi(q) evaluated via the
  square trick phi = (E^T q)^2 with diag contamination removed through
  corrected diag weights wd = KV2dd - Hm @ KVc.

PE array packing (tile_position): kv-side and ans matmuls are (128,64)
col-tiled pairs (2 concurrent M=34 matmuls, one PSUM bank); u = E^T q runs
as (32,64) 8-way tiles (4 a-chunks x 2 feature halves) into a 4-bank PSUM
mega tile. Squares u^2 drain via wide-FD ScalarE ACTIVATE (plus a DVE
copy+mul path) into bf16 phit. Weight prep (p2) uses regular (128,64)
matmuls against a stacked identity, which also merges the two col-tile
partial sums. Interleaved accumulation chains in one bank use start=True
only on the very first matmul of the bank (has_written bits give
overwrite-on-first-touch for the other chain). Host pretransposes q (qt4,
qd incl. A2 q^2 rows) and lays out k with a 16-col wraparound duplicate
for stride-1 DVE products.
"""
import dataclasses
import numpy as np

import concourse.bass as bass
import concourse.tile as tile
from concourse import mybir, bacc
from concourse.bass_utils import run_bass_kernel_spmd

F32 = mybir.dt.float32
BF16 = mybir.dt.bfloat16
SQ = mybir.ActivationFunctionType.Square

A0, A1, A2 = 1.0, 1.0, 0.5
B, H, N, D = 2, 16, 4096, 32
NCORES = 8
BH_PER_CORE = (B * H) // NCORES   # 4
NT = N // 128                     # 32 n-tiles
E1 = 34                           # 32 num + 1 den + 1 pad
NOFF = 496                        # 480 (gaps 1..15) + 16 (half gap 16)

# kt feature-column layout (562 cols):
#  [0:32 k | 32:512 off gaps1-15 | 512:528 half16 | 528:529 ones |
#   529:561 diag | 561:562 pad]
KA_W, KB_W = 288, 274  # ka = kt cols 0:288, kb = kt cols 288:562

# square waves (s,h) routed to the DVE copy+mul path instead of ScalarE
SQ_ON_DVE = {(3, 0), (3, 1), (2, 1)}


def _ap_free(x: bass.AP, free_ap, extra_offset=0):
    return dataclasses.replace(
        x, offset=x.offset + extra_offset,
        ap=[x.ap[0]] + [list(p) for p in free_ap])


def _pairs():
    ps = []
    for g in range(1, 16):
        for d in range(D):
            ps.append((d, (d + g) % D))
    for d in range(16):
        ps.append((d, d + 16))
    return ps


def _host_consts():
    import ml_dtypes
    P = _pairs()
    beta = np.sqrt(A2)
    E = np.zeros((D, NOFF), np.float32)
    Hm = np.zeros((D, NOFF), np.float32)
    for f, (d1, d2) in enumerate(P):
        E[d1, f] += beta
        E[d2, f] += beta
        Hm[d1, f] += 1.0
        Hm[d2, f] += 1.0
    erep = np.zeros((128, 512), np.float32)
    for a in range(4):
        erep[32 * a:32 * a + 32, 0:NOFF] = E
    # hmtT[f', s, 32+d] = Hm[d, 128*s + f']  (d lands at rows 32:64 of out)
    hmtT = np.zeros((128, 4, 64), np.float32)
    for s in range(4):
        w = min(128, NOFF - 128 * s)
        hmtT[0:w, s, 32:64] = Hm[:, 128 * s:128 * s + w].T
    # isum [128, 64]: rows 0:34 I34, rows 64:98 I34 (merges col-tile halves)
    isum = np.zeros((128, 64), np.float32)
    for m in range(E1):
        isum[m, m] = 1.0
        isum[64 + m, m] = 1.0
    return (erep.astype(ml_dtypes.bfloat16), hmtT.astype(ml_dtypes.bfloat16),
            isum.astype(ml_dtypes.bfloat16))


def build_nc():
    nc = bacc.Bacc(None, target_bir_lowering=False)

    qt4_in = nc.declare_dram_parameter("qt4", [BH_PER_CORE, 128, 1024], BF16,
                                       isOutput=False)
    qd_in = nc.declare_dram_parameter("qd", [BH_PER_CORE, 128, 2, 1024], BF16,
                                      isOutput=False)
    kpr_in = nc.declare_dram_parameter("kpr", [BH_PER_CORE, 128, NT, 48], BF16,
                                       isOutput=False)
    vv_in = nc.declare_dram_parameter("vv", [BH_PER_CORE, 128, NT, 32], BF16,
                                      isOutput=False)
    erep_in = nc.declare_dram_parameter("erep", [128, 512], BF16,
                                        isOutput=False)
    hmt_in = nc.declare_dram_parameter("hmt", [128, 4, 64], BF16,
                                       isOutput=False)
    isum_in = nc.declare_dram_parameter("isum", [128, 64], BF16,
                                        isOutput=False)
    ones_in = nc.declare_dram_parameter("ones1", [16, 4, 1024], BF16,
                                        isOutput=False)
    out = nc.declare_dram_parameter("out", [BH_PER_CORE, 2, 98, 1024], F32,
                                    isOutput=True)

    with tile.TileContext(nc) as tc:
        with (
            tc.tile_pool(name="sb_const", bufs=1) as sb_const,
            tc.tile_pool(name="sb_q", bufs=2) as sb_q,
            tc.tile_pool(name="sb_kp", bufs=2) as sb_kp,
            tc.tile_pool(name="sb_w", bufs=2) as sb_w,
            tc.tile_pool(name="sb_ucp", bufs=2) as sb_ucp,
            tc.tile_pool(name="sb_ev", bufs=2) as sb_ev,
            tc.tile_pool(name="ps_kv", bufs=1, space="PSUM") as ps_kv,
            tc.tile_pool(name="ps_mega", bufs=1, space="PSUM") as ps_mega,
            tc.tile_pool(name="ps_ans", bufs=1, space="PSUM") as ps_ans,
        ):
            # ---- constants ----
            erep = sb_const.tile([128, 512], BF16)
            nc.sync.dma_start(out=erep[:], in_=erep_in[:])
            hmt = sb_const.tile([128, 4, 64], BF16)
            nc.sync.dma_start(out=hmt[:], in_=hmt_in[:])
            isum = sb_const.tile([128, 64], BF16)
            nc.sync.dma_start(out=isum[:], in_=isum_in[:])

            kts = [sb_const.tile([128, 8, 562], BF16, name=f"kts{i}")
                   for i in range(3)]
            vxs = [sb_const.tile([128, 8, E1], BF16, name=f"vxs{i}")
                   for i in range(3)]
            for i in range(3):
                nc.gpsimd.memset(kts[i][:, :, 528:529], 1.0)
                nc.gpsimd.memset(kts[i][:, :, 561:562], 0.0)
                nc.gpsimd.memset(vxs[i][:, :, 32:34], 1.0)
            phits = [sb_const.tile([128, 4, 4, 1024], BF16, name=f"phit{i}")
                     for i in range(2)]
            for i in range(2):
                nc.sync.dma_start(out=phits[i][112:128, :, 3, :],
                                  in_=ones_in[:])
            wqd_ev = sb_const.tile([128, E1], BF16, name="wqd_ev")
            wqd_od = sb_const.tile([128, E1], BF16, name="wqd_od")
            nc.gpsimd.memset(wqd_ev[:], 0.0)
            nc.gpsimd.memset(wqd_od[:], 0.0)

            def ktvx(b, g):
                i = (4 * b + g) % 3
                return kts[i], vxs[i]

            def emit_dma_in(b):
                st = dict(b=b, phit=phits[b % 2])
                st["qt4"] = sb_q.tile([128, 1024], BF16, tag="qt4",
                                      name=f"qt4_{b}")
                nc.sync.dma_start(out=st["qt4"][:], in_=qt4_in[b])
                st["qd"] = sb_q.tile([128, 2, 1024], BF16, tag="qd",
                                     name=f"qd_{b}")
                nc.sync.dma_start(out=st["qd"][:], in_=qd_in[b])
                st["kpr"] = sb_kp.tile([128, NT, 48], BF16, tag="kpr",
                                       name=f"kpr_{b}")
                nc.sync.dma_start(out=st["kpr"][:], in_=kpr_in[b])
                st["ka"] = ps_kv.tile([128, KA_W], F32, tag="ka",
                                      name=f"ka{b}")
                st["kb"] = ps_kv.tile([128, KB_W], F32, tag="kb",
                                      name=f"kb{b}")
                return st

            def emit_p1_dma(b, g):
                kt, vx = ktvx(b, g)
                ts8 = slice(8 * g, 8 * g + 8)
                nc.sync.dma_start(out=kt[:, :, 0:32],
                                  in_=kpr_in[b][:, ts8, 0:32])
                nc.sync.dma_start(out=vx[:, :, 0:32], in_=vv_in[b][:, ts8, :])

            def emit_p1_group(cur, g):
                b = cur["b"]
                kt, vx = ktvx(b, g)
                ts8 = slice(8 * g, 8 * g + 8)
                kp = cur["kpr"]
                # off-diag products, gaps 1..15 -> kt cols 32:512
                nc.vector.tensor_mul(
                    _ap_free(kt[:, 0, 32:512], [[562, 8], [32, 15], [1, 32]]),
                    _ap_free(kp[:, 8 * g, 0:32], [[48, 8], [0, 15], [1, 32]]),
                    _ap_free(kp[:, 8 * g, 1:33], [[48, 8], [1, 15], [1, 32]]))
                # half gap 16 -> cols 512:528 ; diag -> cols 529:561
                nc.vector.tensor_mul(kt[:, :, 512:528], kp[:, ts8, 0:16],
                                     kp[:, ts8, 16:32])
                nc.vector.tensor_mul(kt[:, :, 529:561], kp[:, ts8, 0:32],
                                     kp[:, ts8, 0:32])
                for tt in range(8):
                    t = 8 * g + tt
                    par = (t % 2) * 64
                    st_, sp = (t < 2), (t >= NT - 2)
                    nc.tensor.matmul(cur["ka"][par:par + E1, :],
                                     vx[:, tt, :], kt[:, tt, 0:KA_W],
                                     start=st_, stop=sp,
                                     skip_group_check=True)
                    nc.tensor.matmul(cur["kb"][par:par + E1, :],
                                     vx[:, tt, :], kt[:, tt, KA_W:562],
                                     start=st_, stop=sp,
                                     skip_group_check=True)

            def emit_u_group(cur, s, h):
                b = cur["b"]
                mega = ps_mega.tile([128, 2048], F32, tag="mega",
                                    name=f"mega{b}_{s}_{h}")
                qt4 = cur["qt4"]
                hs = slice(512 * h, 512 * h + 512)
                for a in range(4):
                    for fh in range(2):
                        fw = 64 if (s < 3 or fh == 0) else 48
                        fc = 128 * s + 64 * fh
                        nc.tensor.matmul(
                            mega[64 * fh:64 * fh + fw,
                                 512 * a:512 * a + 512],
                            erep[32 * a:32 * a + 32, fc:fc + fw],
                            qt4[32 * a:32 * a + 32, hs],
                            start=True, stop=True,
                            tile_position=(32 * a, 64 * fh),
                            skip_group_check=True)
                pp = 128 if s < 3 else 112
                phit = cur["phit"]
                dst = _ap_free(phit[0:pp, 0, s, 512 * h:512 * h + 512],
                               [[4096, 4], [1, 512]])
                src = _ap_free(mega[0:pp, 0:512], [[512, 4], [1, 512]])
                if (s, h) in SQ_ON_DVE:
                    ucp = sb_ucp.tile([128, 2048], BF16, tag="ucp",
                                      name=f"ucp{b}_{s}_{h}")
                    nc.vector.tensor_copy(ucp[0:pp, :], mega[0:pp, :])
                    us = _ap_free(ucp[0:pp, 0:512], [[512, 4], [1, 512]])
                    nc.vector.tensor_mul(dst, us, us)
                else:
                    nc.scalar.activation(out=dst, in_=src, func=SQ, scale=1.0)

            def emit_ans_chunk(prev, pair, h):
                b = prev["b"]
                if h == 0:
                    prev["ansm"] = ps_ans.tile([128, 1024], F32, tag="ansm",
                                               name=f"ansm{b}_{pair}")
                ansm = prev["ansm"]
                hs = slice(512 * h, 512 * h + 512)
                phit = prev["phit"]
                wc = prev["wc"]
                for s in range(4):
                    for ci in range(2):
                        a = 2 * pair + ci
                        nc.tensor.matmul(
                            ansm[64 * ci:64 * ci + E1, hs],
                            wc[:, 34 * s:34 * s + 34], phit[:, a, s, hs],
                            start=(s == 0), stop=False,
                            skip_group_check=True)
                for ci in range(2):
                    nc.tensor.matmul(
                        ansm[64 * ci:64 * ci + E1, hs],
                        wqd_ev if ci == 0 else wqd_od,
                        prev["qd"][:, pair, hs],
                        start=False, stop=(ci == 1), skip_group_check=True)

            def emit_evict(prev, pair):
                b = prev["b"]
                ev = sb_ev.tile([128, 1024], F32, tag="ev",
                                name=f"ev{b}_{pair}")
                nc.vector.tensor_copy(ev[0:98, :], prev["ansm"][0:98, :])
                nc.sync.dma_start(out=out[b][pair], in_=ev[0:98, :])

            def emit_p2(cur):
                b = cur["b"]
                cov = sb_w.tile([128, KA_W], BF16, tag="cov", name=f"cov{b}")
                nc.vector.tensor_copy(cov[:], cur["ka"][:])
                cov2 = sb_w.tile([128, KB_W], BF16, tag="cov2",
                                 name=f"cov2{b}")
                nc.vector.tensor_copy(cov2[:], cur["kb"][:])

                wcp = ps_ans.tile([128, 1024], F32, tag="ansm",
                                  name=f"wcp{b}")
                # wc chunks (merged col-tile halves via isum):
                srcs = [(cov, 32), (cov, 96), (cov, 160), (cov, 224),
                        (cov2, 0), (cov2, 64), (cov2, 128), (cov2, 192)]
                for i, (sc, c0) in enumerate(srcs):
                    s_, fh = i // 2, i % 2
                    w = 49 if i == 7 else 64
                    nc.tensor.matmul(wcp[64 * fh:64 * fh + w,
                                         34 * s_:34 * s_ + 34],
                                     sc[:, c0:c0 + w], isum[:, 0:34],
                                     start=True, stop=False,
                                     skip_group_check=True)
                # wq rows 0:32 at cols 170:204 ; diagT rows 32:64 at 204:238
                nc.tensor.matmul(wcp[0:64, 170:204], cov[:, 0:64],
                                 isum[:, 0:34], start=True, stop=False,
                                 skip_group_check=True)
                nc.tensor.matmul(wcp[0:64, 204:238], cov2[:, 209:273],
                                 isum[:, 0:34], start=True, stop=False,
                                 skip_group_check=True)
                wc = sb_w.tile([128, 136], BF16, tag="wc", name=f"wc{b}")
                nc.scalar.copy(out=wc[:], in_=wcp[:, 0:136])
                # hkT (rows 32:64 of out) = Hm @ KVc, accumulated over s
                for s_ in range(4):
                    nc.tensor.matmul(wcp[0:64, 136:170], hmt[:, s_, :],
                                     wc[:, 34 * s_:34 * s_ + 34],
                                     start=(s_ == 0), stop=(s_ == 3),
                                     skip_group_check=True)
                hks = sb_w.tile([64, E1], BF16, tag="hks", name=f"hks{b}")
                nc.scalar.copy(out=hks[:], in_=wcp[0:64, 136:170])
                wdt = sb_w.tile([64, E1], BF16, tag="wdt", name=f"wdt{b}")
                nc.vector.scalar_tensor_tensor(
                    out=wdt[32:64, :], in0=wcp[32:64, 204:238], scalar=1.0,
                    in1=hks[32:64, :],
                    op0=mybir.AluOpType.mult, op1=mybir.AluOpType.subtract)
                wqs = sb_w.tile([32, E1], BF16, tag="wqs", name=f"wqs{b}")
                nc.scalar.copy(out=wqs[:], in_=wcp[0:32, 170:204])
                # assemble wqd masks (partition moves via DMA)
                nc.scalar.copy(out=wqd_ev[0:32, :], in_=wcp[0:32, 170:204])
                nc.sync.dma_start(out=wqd_od[32:64, :], in_=wqs[:])
                nc.sync.dma_start(out=wqd_ev[64:96, :], in_=wdt[32:64, :])
                nc.sync.dma_start(out=wqd_od[96:128, :], in_=wdt[32:64, :])
                cur["wc"] = wc

            # ---------------- main pipeline over b ----------------
            prev = None
            nxt = emit_dma_in(0)
            emit_p1_dma(0, 0)
            for b in range(BH_PER_CORE + 1):
                cur = nxt if b < BH_PER_CORE else None
                nxt = None
                for gi in range(8):
                    g, h = gi // 2, gi % 2
                    if cur is not None and h == 0:
                        if g + 1 < 4:
                            emit_p1_dma(b, g + 1)
                        elif b + 1 < BH_PER_CORE:
                            emit_p1_dma(b + 1, 0)
                        emit_p1_group(cur, g)
                    if prev is not None and h == 1:
                        c = gi // 2
                        emit_ans_chunk(prev, c // 2, c % 2)
                        if c % 2 == 1:
                            emit_evict(prev, c // 2)
                    if cur is not None:
                        emit_u_group(cur, g, h)
                    if cur is not None and gi == 3 and b + 1 < BH_PER_CORE:
                        nxt = emit_dma_in(b + 1)
                if cur is not None:
                    emit_p2(cur)
                prev = cur

    nc.compile()
    return nc


_NC_CACHE = None


def _get_nc():
    global _NC_CACHE
    if _NC_CACHE is None:
        _NC_CACHE = build_nc()
    return _NC_CACHE


def _in_maps(q, k, v):
    import ml_dtypes
    BH = B * H
    qf = q.reshape(BH, N, D)
    kf = k.reshape(BH, N, D)
    vf = v.reshape(BH, N, D)
    qt = qf.reshape(BH, 4, 1024, D).transpose(0, 1, 3, 2)  # [bh, a, d, 1024]
    qt4 = np.ascontiguousarray(qt.reshape(BH, 128, 1024)).astype(
        ml_dtypes.bfloat16)
    qd = np.empty((BH, 128, 2, 1024), np.float32)
    for p in range(2):
        qd[:, 0:32, p, :] = qt[:, 2 * p]
        qd[:, 32:64, p, :] = qt[:, 2 * p + 1]
        qd[:, 64:96, p, :] = A2 * qt[:, 2 * p] ** 2
        qd[:, 96:128, p, :] = A2 * qt[:, 2 * p + 1] ** 2
    qd = np.ascontiguousarray(qd).astype(ml_dtypes.bfloat16)
    kk = kf.reshape(BH, NT, 128, D).transpose(0, 2, 1, 3)  # [bh, p, t, d]
    kpr = np.ascontiguousarray(
        np.concatenate([kk, kk[:, :, :, 0:16]], axis=3)).astype(
            ml_dtypes.bfloat16)
    vvt = np.ascontiguousarray(
        vf.reshape(BH, NT, 128, D).transpose(0, 2, 1, 3)).astype(
            ml_dtypes.bfloat16)
    erep, hmtT, isum = _host_consts()
    ones16 = np.zeros((16, 4, 1024), np.float32)
    ones16[0] = 1.0
    ones16 = ones16.astype(ml_dtypes.bfloat16)
    in_maps = []
    for c in range(NCORES):
        sl = slice(c * BH_PER_CORE, (c + 1) * BH_PER_CORE)
        in_maps.append({
            "qt4": np.ascontiguousarray(qt4[sl]),
            "qd": np.ascontiguousarray(qd[sl]),
            "kpr": np.ascontiguousarray(kpr[sl]),
            "vv": np.ascontiguousarray(vvt[sl]),
            "erep": erep, "hmt": hmtT, "isum": isum,
            "ones1": ones16,
        })
    return in_maps


def _postprocess(res):
    outs = [res.results[c]["out"] for c in range(NCORES)]
    o = np.stack(outs, 0).reshape(B * H, 2, 98, 1024)
    ans = np.empty((B * H, 4, E1, 1024), np.float32)
    ans[:, 0::2] = o[:, :, 0:E1, :]
    ans[:, 1::2] = o[:, :, 64:64 + E1, :]
    num = ans[:, :, 0:D, :]
    den = ans[:, :, D:D + 1, :]
    r = (num / den).transpose(0, 1, 3, 2)      # [bh, a, 1024, d]
    return np.ascontiguousarray(r.reshape(B, H, N, D)).astype(np.float32)


def run_traced(q, k, v):
    q = np.ascontiguousarray(np.asarray(q, dtype=np.float32))
    k = np.ascontiguousarray(np.asarray(k, dtype=np.float32))
    v = np.ascontiguousarray(np.asarray(v, dtype=np.float32))
    nc = _get_nc()
    try:
        return run_bass_kernel_spmd(nc, _in_maps(q, k, v),
                                    core_ids=list(range(NCORES)), trace=True)
    except Exception as e:
        print("traced run failed:", e)
        return None


def kernel(q, k, v):
    q = np.ascontiguousarray(np.asarray(q, dtype=np.float32))
    k = np.ascontiguousarray(np.asarray(k, dtype=np.float32))
    v = np.ascontiguousarray(np.asarray(v, dtype=np.float32))
    assert q.shape == (B, H, N, D)
    nc = _get_nc()
    res = run_bass_kernel_spmd(nc, _in_maps(q, k, v),
                               core_ids=list(range(NCORES)))
    return _postprocess(res)


if __name__ == "__main__":
    rng = np.random.default_rng(0)
    q = rng.standard_normal((B, H, N, D), dtype=np.float32)
    k = rng.standard_normal((B, H, N, D), dtype=np.float32)
    v = rng.standard_normal((B, H, N, D), dtype=np.float32)
    o = kernel(q, k, v)
    print("ran", o.shape, o.dtype)
